# revision 7
# baseline (speedup 1.0000x reference)
"""DigitCaps dynamic-routing kernel for Trainium2 (8 NeuronCores, Bass/Tile).

Math (per routing iteration, reformulated to avoid materializing u_hat):
    u_hat[b,i,j,u] = sum_k W[i,j,u,k] * x[b,k,i]
    s[b,ju]  = sum_{ki} X[ki,b] * (c[i,j] * W[ki,ju])          (PE matmul, K=9216)
    v        = squash(s)  with the reference's quirky j-axis norm
    G[ki,ju] = sum_b X[b,ki] * v[b,ju]                         (PE matmul, K=64)
    b[i,j]   = sum_{k,u} W[ki,ju] * G[ki,ju]                   (DVE STT w/ accum)
    b is AllReduduced (sum) over the 8 cores each iteration (batch mean).

Sharding: data-parallel over batch B=512 -> 64 rows per core; W replicated.
"""

import sys

sys.path.insert(0, "/opt/trn_rl_repo")

from contextlib import ExitStack

import numpy as np

B = 512
NCORES = 8
BL = B // NCORES  # 64 local batch rows
K = 8             # in_units (primary capsule dim)
IC = 1152         # in_channels (number of primary capsules)
J = 10            # num_units (output capsules)
U = 16            # unit_size
JU = J * U        # 160
NT = IC // 128    # 9 i-chunks of 128
NKT = K * NT      # 72 ki-chunks of 128
BETA = 1.45
NUM_ROUTING = 3

_CACHE = {}


def _build_nc():
    import concourse.bass as bass
    import concourse.tile as tile
    from concourse import bacc, mybir
    from concourse.masks import make_identity

    f32 = mybir.dt.float32
    Alu = mybir.AluOpType
    Act = mybir.ActivationFunctionType

    nc = bacc.Bacc("TRN2", target_bir_lowering=False, debug=False,
                   num_devices=NCORES)

    xs = nc.dram_tensor("xs", [BL, K, IC], f32, kind="ExternalInput").ap()
    w = nc.dram_tensor("w", [IC, J, U, K], f32, kind="ExternalInput").ap()
    out = nc.dram_tensor("out", [BL, J, 4, 4], f32, kind="ExternalOutput").ap()

    xs_flat = xs.rearrange("b k i -> b (k i)")          # [64, 9216]
    w_r = w.rearrange("(t p) j u k -> p t (j u k)", p=128)  # [128, 9, 1280]
    out_flat = out.rearrange("b j g h -> b (j g h)")    # [64, 160]

    with tile.TileContext(nc) as tc, ExitStack() as ctx:
        consts = ctx.enter_context(tc.tile_pool(name="consts", bufs=1))
        small = ctx.enter_context(tc.tile_pool(name="small", bufs=2))
        scratch = ctx.enter_context(tc.tile_pool(name="scratch", bufs=8))
        psum = ctx.enter_context(tc.tile_pool(name="psum", bufs=1, space="PSUM"))
        dram = ctx.enter_context(tc.tile_pool(name="dram", bufs=1, space="DRAM"))

        # ---- persistent SBUF tensors ----
        x2 = consts.tile([BL, K * IC], f32)          # x[b, (k i)]
        x1 = consts.tile([128, NKT, BL], f32)        # x^T per ki-chunk
        w_nat = consts.tile([128, NT, J * U * K], f32)  # W natural layout
        wp = consts.tile([128, NKT, JU], f32)        # c-scaled W (matmul rhs)
        crep = consts.tile([128, NT, JU], f32)       # c broadcast over u
        ident = consts.tile([BL, BL], f32)
        ones = consts.tile([128, 128], f32)
        beta_ap = consts.tile([BL, 1], f32)
        nc.vector.memset(beta_ap, BETA)

        # one PSUM tensor = all 8 banks; everything slices into it
        pall = psum.tile([128, K, 512], f32)

        # W as [p, t2, j, u, k] view for strided reads
        w5 = w_nat.rearrange("p t (j u k) -> p t j u k", j=J, u=U)
        # G banks as [p, f, k] view for (u,k) strided reads
        pall_fk = pall.rearrange("p k f -> p f k")

        # ---- loads ----
        for k in range(K):
            nc.sync.dma_start(out=x2[:, k * IC:(k + 1) * IC],
                              in_=xs_flat[:, k * IC:(k + 1) * IC])
        for t2 in range(NT):
            nc.sync.dma_start(out=w_nat[:, t2, :], in_=w_r[:, t2, :])
        make_identity(nc, ident)
        nc.vector.memset(ones, 1.0)

        # ---- build x1 = per-chunk transpose of x2 (PE transpose) ----
        for t in range(NKT):
            ps = pall[:, t % K, :BL]
            nc.tensor.transpose(ps, x2[:, t * 128:(t + 1) * 128], ident)
            nc.scalar.copy(x1[:, t, :], ps)

        # iteration 0 has uniform c = 1/IC
        nc.vector.memset(crep[:, :, :], 1.0 / IC)

        bfulls = {}
        for it in range(NUM_ROUTING):
            if it > 0:
                # ---- softmax over i (given b_full from the AllReduce) ----
                b_full = bfulls[it - 1]
                expb = small.tile([128, NT, J], f32, name=f"expb{it}")
                # exp(b/B): fold the batch-mean 1/B into the exp scale
                nc.scalar.activation(
                    expb.rearrange("p t j -> p (t j)"),
                    b_full.rearrange("p t j -> p (t j)"),
                    Act.Exp, scale=1.0 / B)
                # Z[j] = sum_i exp(b[i,j]), broadcast to 128 partitions via
                # an accumulating ones-matmul; bank 7 of PSUM
                zp = pall[:, K - 1, :J]
                for t2 in range(NT):
                    nc.tensor.matmul(zp, ones, expb[:, t2, :],
                                     start=(t2 == 0), stop=(t2 == NT - 1))
                zinv = small.tile([128, J], f32, name=f"zinv{it}")
                nc.vector.reciprocal(zinv, zp)
                # crep[i, (j,u)] = expb[i,j] * zinv[j]  (broadcast over u)
                for t2 in range(NT):
                    nc.vector.tensor_mul(
                        crep[:, t2, :].rearrange("p (j u) -> p j u", j=J),
                        expb[:, t2, :].unsqueeze(-1).broadcast_to([128, J, U]),
                        zinv.unsqueeze(-1).broadcast_to([128, J, U]))

            # ---- wp = crep * W  (72 DVE multiplies) ----
            for t in range(NKT):
                k, t2 = divmod(t, NT)
                nc.vector.tensor_mul(
                    wp[:, t, :].rearrange("p (j u) -> p j u", j=J),
                    w5[:, t2, :, :, k],
                    crep[:, t2, :].rearrange("p (j u) -> p j u", j=J))

            # ---- s = X1^T @ wp : accumulate 72 chunks into PSUM bank 0 ----
            sp = pall[:BL, 0, :JU]
            for t in range(NKT):
                nc.tensor.matmul(sp, x1[:, t, :], wp[:, t, :],
                                 start=(t == 0), stop=(t == NKT - 1))

            # ---- squash (reference quirk: norm over the j axis per (b,u)) ----
            ssq = small.tile([BL, JU], f32, name=f"ssq{it}")
            nc.scalar.activation(ssq, sp, Act.Square)
            msq = small.tile([BL, U], f32, name=f"msq{it}")
            nc.vector.tensor_reduce(
                msq, ssq.rearrange("b (j u) -> b u j", j=J),
                axis=mybir.AxisListType.X, op=Alu.add)
            mag = small.tile([BL, U], f32, name=f"mag{it}")
            tpb = small.tile([BL, U], f32, name=f"tpb{it}")
            rin = small.tile([BL, U], f32, name=f"rin{it}")
            fv = small.tile([BL, U], f32, name=f"fv{it}")
            nc.scalar.activation(mag, msq, Act.Sqrt)
            nc.scalar.activation(tpb, msq, Act.Identity, bias=beta_ap[:, :])
            nc.vector.reciprocal(rin, tpb)
            nc.vector.tensor_mul(fv, mag, rin)
            v = small.tile([BL, JU], f32, name=f"v{it}")
            nc.vector.tensor_mul(
                v.rearrange("b (j u) -> b j u", j=J),
                sp.rearrange("b (j u) -> b j u", j=J),
                fv.unsqueeze(1).broadcast_to([BL, J, U]))

            if it == NUM_ROUTING - 1:
                nc.sync.dma_start(out=out_flat, in_=v)
                continue

            # ---- G = X2^T-chunks @ v, per (t2): 8 banks; then b-update ----
            b_part = small.tile([128, NT, J], f32, name=f"bpart{it}")
            for t2 in range(NT):
                for k in range(K):
                    nc.tensor.matmul(
                        pall[:, k, :JU],
                        x2[:, (k * NT + t2) * 128:(k * NT + t2) * 128 + 128],
                        v, start=True, stop=True)
                for j in range(J):
                    so = scratch.tile([128, U, K], f32, name="stt_scratch")
                    nc.vector.scalar_tensor_tensor(
                        out=so,
                        in0=w5[:, t2, j, :, :],
                        scalar=1.0,
                        in1=pall_fk[:, j * U:(j + 1) * U, :],
                        op0=Alu.mult, op1=Alu.mult,
                        accum_out=b_part[:, t2, j:j + 1])

            # ---- AllReduce b over the 8 cores ----
            cc_in = dram.tile([IC, J], f32, name=f"ccin{it}")
            cc_out = dram.tile([IC, J], f32, name=f"ccout{it}",
                               addr_space="Shared")
            cc_in_r = cc_in.rearrange("(t p) j -> p t j", p=128)
            cc_out_r = cc_out.rearrange("(t p) j -> p t j", p=128)
            nc.sync.dma_start(out=cc_in_r, in_=b_part)
            nc.gpsimd.collective_compute(
                "AllReduce", Alu.add,
                replica_groups=[list(range(NCORES))],
                ins=[cc_in[:, :]], outs=[cc_out[:, :]])
            b_full = small.tile([128, NT, J], f32, name=f"bfull{it}")
            nc.sync.dma_start(out=b_full, in_=cc_out_r)
            bfulls[it] = b_full

    nc.compile()
    return nc


def _get_nc():
    if "nc" not in _CACHE:
        _CACHE["nc"] = _build_nc()
    return _CACHE["nc"]


def _run(x, W, trace=False, **kw):
    from concourse import bass_utils

    nc = _get_nc()
    x = np.ascontiguousarray(np.asarray(x, dtype=np.float32))
    W = np.ascontiguousarray(np.asarray(W, dtype=np.float32))
    in_maps = [
        {"xs": x[c * BL:(c + 1) * BL], "w": W}
        for c in range(NCORES)
    ]
    res = bass_utils.run_bass_kernel_spmd(
        nc, in_maps, core_ids=list(range(NCORES)), trace=trace, **kw)
    outs = [res.results[c]["out"] for c in range(NCORES)]
    full = np.concatenate(outs, axis=0).reshape(B, J, 4, U // 4)
    return full, res


def kernel(x, W):
    full, _ = _run(x, W, trace=False)
    return full


# revision 9
# speedup vs baseline: 1.1082x; 1.1082x over previous
"""DigitCaps dynamic-routing kernel for Trainium2 (8 NeuronCores, Bass/Tile).

Math (per routing iteration, reformulated to avoid materializing u_hat):
    u_hat[b,i,j,u] = sum_k W[i,j,u,k] * x[b,k,i]
    s[b,ju]  = sum_{ki} X[ki,b] * (c[i,j] * W[ki,ju])          (PE matmul, K=9216)
    v        = squash(s)  with the reference's quirky j-axis norm
    G[ki,ju] = sum_b X[b,ki] * v[b,ju]                         (PE matmul, K=64)
    b[i,j]   = sum_{k,u} W[ki,ju] * G[ki,ju]                   (DVE STT w/ accum)
    b is AllReduduced (sum) over the 8 cores each iteration (batch mean).

Sharding: data-parallel over batch B=512 -> 64 rows per core; W replicated.
"""

import sys

sys.path.insert(0, "/opt/trn_rl_repo")

from contextlib import ExitStack

import numpy as np

B = 512
NCORES = 8
BL = B // NCORES  # 64 local batch rows
K = 8             # in_units (primary capsule dim)
IC = 1152         # in_channels (number of primary capsules)
J = 10            # num_units (output capsules)
U = 16            # unit_size
JU = J * U        # 160
NT = IC // 128    # 9 i-chunks of 128
NKT = K * NT      # 72 ki-chunks of 128
BETA = 1.45
NUM_ROUTING = 3

_CACHE = {}


def _build_nc():
    import concourse.bass as bass
    import concourse.tile as tile
    from concourse import bacc, mybir
    from concourse.masks import make_identity

    f32 = mybir.dt.float32
    Alu = mybir.AluOpType
    Act = mybir.ActivationFunctionType

    nc = bacc.Bacc("TRN2", target_bir_lowering=False, debug=False,
                   num_devices=NCORES)

    xs = nc.dram_tensor("xs", [BL, K, IC], f32, kind="ExternalInput").ap()
    w = nc.dram_tensor("w", [IC, J, U, K], f32, kind="ExternalInput").ap()
    out = nc.dram_tensor("out", [BL, J, 4, 4], f32, kind="ExternalOutput").ap()

    xs_flat = xs.rearrange("b k i -> b (k i)")          # [64, 9216]
    w_r = w.rearrange("(t p) j u k -> p t (j u k)", p=128)  # [128, 9, 1280]
    out_flat = out.rearrange("b j g h -> b (j g h)")    # [64, 160]

    with tile.TileContext(nc) as tc, ExitStack() as ctx:
        consts = ctx.enter_context(tc.tile_pool(name="consts", bufs=1))
        small = ctx.enter_context(tc.tile_pool(name="small", bufs=2))
        scratch = ctx.enter_context(tc.tile_pool(name="scratch", bufs=8))
        psum = ctx.enter_context(tc.tile_pool(name="psum", bufs=1, space="PSUM"))
        dram = ctx.enter_context(tc.tile_pool(name="dram", bufs=1, space="DRAM"))

        # ---- persistent SBUF tensors ----
        x2 = consts.tile([BL, K * IC], f32)          # x[b, (k i)]
        x1 = consts.tile([128, NKT, BL], f32)        # x^T per ki-chunk
        w_nat = consts.tile([128, NT, J * U * K], f32)  # W natural layout
        wp = consts.tile([128, NKT, JU], f32)        # c-scaled W (matmul rhs)
        crep = consts.tile([128, NT, JU], f32)       # c broadcast over u
        ident = consts.tile([BL, BL], f32)
        ones = consts.tile([128, 128], f32)
        beta_ap = consts.tile([BL, 1], f32)
        nc.vector.memset(beta_ap, BETA)

        # one PSUM tensor = all 8 banks; everything slices into it
        pall = psum.tile([128, K, 512], f32)

        # W as [p, t2, j, u, k] view for strided reads
        w5 = w_nat.rearrange("p t (j u k) -> p t j u k", j=J, u=U)
        # G banks as [p, f, k] view for (u,k) strided reads
        pall_fk = pall.rearrange("p k f -> p f k")

        # ---- loads: split into small pieces so transfers spread across the
        # 16 DMA engines (a single transfer lands on one engine @~22GB/s) ----
        HIC = IC // 2
        for k in range(K):
            for h in range(2):
                nc.sync.dma_start(
                    out=x2[:, k * IC + h * HIC:k * IC + (h + 1) * HIC],
                    in_=xs_flat[:, k * IC + h * HIC:k * IC + (h + 1) * HIC])
            if k + 1 < NT:
                t2 = k
                for h in range(2):
                    nc.sync.dma_start(out=w_nat[:, t2, h * 640:(h + 1) * 640],
                                      in_=w_r[:, t2, h * 640:(h + 1) * 640])
        for h in range(2):
            nc.sync.dma_start(out=w_nat[:, NT - 1, h * 640:(h + 1) * 640],
                              in_=w_r[:, NT - 1, h * 640:(h + 1) * 640])
        make_identity(nc, ident)
        nc.vector.memset(ones, 1.0)

        # ---- build x1 = per-chunk transpose of x2 (PE transpose) ----
        # evacuation alternates ACT/DVE so neither engine paces the PE
        for t in range(NKT):
            ps = pall[:, t % K, :BL]
            nc.tensor.transpose(ps, x2[:, t * 128:(t + 1) * 128], ident)
            if t % 2 == 0:
                nc.scalar.copy(x1[:, t, :], ps)
            else:
                nc.vector.tensor_copy(x1[:, t, :], ps)

        # iteration 0 has uniform c = 1/IC
        nc.vector.memset(crep[:, :, :], 1.0 / IC)

        bfulls = {}
        for it in range(NUM_ROUTING):
            if it > 0:
                # ---- softmax over i (given b_full from the AllReduce) ----
                b_full = bfulls[it - 1]
                expb = small.tile([128, NT, J], f32, name=f"expb{it}")
                # exp(b/B): fold the batch-mean 1/B into the exp scale
                nc.scalar.activation(
                    expb.rearrange("p t j -> p (t j)"),
                    b_full.rearrange("p t j -> p (t j)"),
                    Act.Exp, scale=1.0 / B)
                # Z[j] = sum_i exp(b[i,j]), broadcast to 128 partitions via
                # an accumulating ones-matmul; bank 7 of PSUM
                zp = pall[:, K - 1, :J]
                for t2 in range(NT):
                    nc.tensor.matmul(zp, ones, expb[:, t2, :],
                                     start=(t2 == 0), stop=(t2 == NT - 1))
                zinv = small.tile([128, J], f32, name=f"zinv{it}")
                nc.vector.reciprocal(zinv, zp)
                # crep[i, (j,u)] = expb[i,j] * zinv[j]  (broadcast over u)
                for t2 in range(NT):
                    nc.vector.tensor_mul(
                        crep[:, t2, :].rearrange("p (j u) -> p j u", j=J),
                        expb[:, t2, :].unsqueeze(-1).broadcast_to([128, J, U]),
                        zinv.unsqueeze(-1).broadcast_to([128, J, U]))

            # ---- wp = crep * W  (72 DVE multiplies) ----
            for t in range(NKT):
                k, t2 = divmod(t, NT)
                nc.vector.tensor_mul(
                    wp[:, t, :].rearrange("p (j u) -> p j u", j=J),
                    w5[:, t2, :, :, k],
                    crep[:, t2, :].rearrange("p (j u) -> p j u", j=J))

            # ---- s = X1^T @ wp : accumulate 72 chunks into PSUM bank 0 ----
            sp = pall[:BL, 0, :JU]
            for t in range(NKT):
                nc.tensor.matmul(sp, x1[:, t, :], wp[:, t, :],
                                 start=(t == 0), stop=(t == NKT - 1))

            # ---- squash (reference quirk: norm over the j axis per (b,u)) ----
            ssq = small.tile([BL, JU], f32, name=f"ssq{it}")
            nc.scalar.activation(ssq, sp, Act.Square)
            msq = small.tile([BL, U], f32, name=f"msq{it}")
            nc.vector.tensor_reduce(
                msq, ssq.rearrange("b (j u) -> b u j", j=J),
                axis=mybir.AxisListType.X, op=Alu.add)
            mag = small.tile([BL, U], f32, name=f"mag{it}")
            tpb = small.tile([BL, U], f32, name=f"tpb{it}")
            rin = small.tile([BL, U], f32, name=f"rin{it}")
            fv = small.tile([BL, U], f32, name=f"fv{it}")
            nc.scalar.activation(mag, msq, Act.Sqrt)
            nc.scalar.activation(tpb, msq, Act.Identity, bias=beta_ap[:, :])
            nc.vector.reciprocal(rin, tpb)
            nc.vector.tensor_mul(fv, mag, rin)
            v = small.tile([BL, JU], f32, name=f"v{it}")
            nc.vector.tensor_mul(
                v.rearrange("b (j u) -> b j u", j=J),
                sp.rearrange("b (j u) -> b j u", j=J),
                fv.unsqueeze(1).broadcast_to([BL, J, U]))

            if it == NUM_ROUTING - 1:
                nc.sync.dma_start(out=out_flat, in_=v)
                continue

            # ---- G = X2^T-chunks @ v, per (t2): 8 banks; ACT evacuates the
            # whole PSUM round to SBUF so the next round's matmuls don't
            # serialize behind the DVE b-update reads (PSUM bank hazard) ----
            b_part = small.tile([128, NT, J], f32, name=f"bpart{it}")
            for t2 in range(NT):
                for k in range(K):
                    nc.tensor.matmul(
                        pall[:, k, :JU],
                        x2[:, (k * NT + t2) * 128:(k * NT + t2) * 128 + 128],
                        v, start=True, stop=True)
                g_sb = scratch.tile([128, K, JU], f32, name="g_sb", bufs=3)
                nc.scalar.copy(g_sb, pall[:, :, :JU])
                g_fk = g_sb.rearrange("p k f -> p f k")
                for j in range(J):
                    so = scratch.tile([128, U, K], f32, name="stt_scratch")
                    nc.vector.scalar_tensor_tensor(
                        out=so,
                        in0=w5[:, t2, j, :, :],
                        scalar=1.0,
                        in1=g_fk[:, j * U:(j + 1) * U, :],
                        op0=Alu.mult, op1=Alu.mult,
                        accum_out=b_part[:, t2, j:j + 1])

            # ---- AllReduce b over the 8 cores ----
            cc_in = dram.tile([IC, J], f32, name=f"ccin{it}")
            cc_out = dram.tile([IC, J], f32, name=f"ccout{it}",
                               addr_space="Shared")
            cc_in_r = cc_in.rearrange("(t p) j -> p t j", p=128)
            cc_out_r = cc_out.rearrange("(t p) j -> p t j", p=128)
            nc.sync.dma_start(out=cc_in_r, in_=b_part)
            nc.gpsimd.collective_compute(
                "AllReduce", Alu.add,
                replica_groups=[list(range(NCORES))],
                ins=[cc_in[:, :]], outs=[cc_out[:, :]])
            b_full = small.tile([128, NT, J], f32, name=f"bfull{it}")
            nc.sync.dma_start(out=b_full, in_=cc_out_r)
            bfulls[it] = b_full

    nc.compile()
    return nc


def _get_nc():
    if "nc" not in _CACHE:
        _CACHE["nc"] = _build_nc()
    return _CACHE["nc"]


def _run(x, W, trace=False, **kw):
    from concourse import bass_utils

    nc = _get_nc()
    x = np.ascontiguousarray(np.asarray(x, dtype=np.float32))
    W = np.ascontiguousarray(np.asarray(W, dtype=np.float32))
    in_maps = [
        {"xs": x[c * BL:(c + 1) * BL], "w": W}
        for c in range(NCORES)
    ]
    res = bass_utils.run_bass_kernel_spmd(
        nc, in_maps, core_ids=list(range(NCORES)), trace=trace, **kw)
    outs = [res.results[c]["out"] for c in range(NCORES)]
    full = np.concatenate(outs, axis=0).reshape(B, J, 4, U // 4)
    return full, res


def kernel(x, W):
    full, _ = _run(x, W, trace=False)
    return full


# revision 14
# speedup vs baseline: 1.3420x; 1.2109x over previous
"""DigitCaps dynamic-routing kernel for Trainium2 (8 NeuronCores, Bass/Tile).

Math (per routing iteration, reformulated to avoid materializing u_hat):
    u_hat[b,i,j,u] = sum_k W[i,j,u,k] * x[b,k,i]
    s[b,ju]  = sum_{ki} X[ki,b] * (c[i,j] * W[ki,ju])          (PE matmul, K=9216)
    v        = squash(s)  with the reference's quirky j-axis norm
    G[ki,ju] = sum_b X[b,ki] * v[b,ju]                         (PE matmul, K=64)
    b[i,j]   = sum_{k,u} W[ki,ju] * G[ki,ju]                   (DVE STT w/ accum)
    b is AllReduduced (sum) over the 8 cores each iteration (batch mean).

Sharding: data-parallel over batch B=512 -> 64 rows per core; W replicated.
"""

import sys

sys.path.insert(0, "/opt/trn_rl_repo")

from contextlib import ExitStack

import numpy as np

B = 512
NCORES = 8
BL = B // NCORES  # 64 local batch rows
K = 8             # in_units (primary capsule dim)
IC = 1152         # in_channels (number of primary capsules)
J = 10            # num_units (output capsules)
U = 16            # unit_size
JU = J * U        # 160
NT = IC // 128    # 9 i-chunks of 128
NKT = K * NT      # 72 ki-chunks of 128
BETA = 1.45
NUM_ROUTING = 3

_CACHE = {}


def _build_nc():
    import concourse.bass as bass
    import concourse.tile as tile
    from concourse import bacc, mybir
    from concourse.masks import make_identity

    f32 = mybir.dt.float32
    bf16 = mybir.dt.bfloat16
    Alu = mybir.AluOpType
    Act = mybir.ActivationFunctionType

    nc = bacc.Bacc("TRN2", target_bir_lowering=False, debug=False,
                   num_devices=NCORES)

    xs = nc.dram_tensor("xs", [BL, K, IC], f32, kind="ExternalInput").ap()
    w = nc.dram_tensor("w", [IC, J, U, K], f32, kind="ExternalInput").ap()
    out = nc.dram_tensor("out", [BL, J, 4, 4], f32, kind="ExternalOutput").ap()

    xs_flat = xs.rearrange("b k i -> b (k i)")          # [64, 9216]
    w_r = w.rearrange("(t p) j u k -> p t (j u k)", p=128)  # [128, 9, 1280]
    out_flat = out.rearrange("b j g h -> b (j g h)")    # [64, 160]

    with tile.TileContext(nc) as tc, ExitStack() as ctx:
        consts = ctx.enter_context(tc.tile_pool(name="consts", bufs=1))
        small = ctx.enter_context(tc.tile_pool(name="small", bufs=2))
        scratch = ctx.enter_context(tc.tile_pool(name="scratch", bufs=8))
        psum = ctx.enter_context(tc.tile_pool(name="psum", bufs=1, space="PSUM"))
        dram = ctx.enter_context(tc.tile_pool(name="dram", bufs=1, space="DRAM"))

        # ---- persistent SBUF tensors ----
        x2 = consts.tile([BL, K * IC], f32)          # x[b, (k i)]
        x2b = consts.tile([BL, K * IC], bf16)        # bf16 copy for G matmuls
        x1 = consts.tile([128, NKT, BL], f32)        # x^T per ki-chunk
        w_nat = consts.tile([128, NT, J * U * K], f32)  # W natural layout
        wp = consts.tile([128, NKT, JU], f32)        # c-scaled W (matmul rhs)
        crep = consts.tile([128, NT, JU], f32)       # c broadcast over u
        ident = consts.tile([BL, BL], f32)
        ones = consts.tile([128, 128], f32)
        beta_ap = consts.tile([BL, 1], f32)
        nc.vector.memset(beta_ap, BETA)

        # one PSUM tensor = all 8 banks; everything slices into it
        pall = psum.tile([128, K, 512], f32)

        # W as [p, t2, j, u, k] view for strided reads
        w5 = w_nat.rearrange("p t (j u k) -> p t j u k", j=J, u=U)
        # G banks as [p, f, k] view for (u,k) strided reads
        pall_fk = pall.rearrange("p k f -> p f k")

        # ---- loads: split into small pieces so transfers spread across the
        # 16 DMA engines (a single transfer lands on one engine @~22GB/s) ----
        HIC = IC // 2
        for k in range(K):
            for h in range(2):
                nc.sync.dma_start(
                    out=x2[:, k * IC + h * HIC:k * IC + (h + 1) * HIC],
                    in_=xs_flat[:, k * IC + h * HIC:k * IC + (h + 1) * HIC])
            if k + 1 < NT:
                t2 = k
                for h in range(2):
                    nc.sync.dma_start(out=w_nat[:, t2, h * 640:(h + 1) * 640],
                                      in_=w_r[:, t2, h * 640:(h + 1) * 640])
        for h in range(2):
            nc.sync.dma_start(out=w_nat[:, NT - 1, h * 640:(h + 1) * 640],
                              in_=w_r[:, NT - 1, h * 640:(h + 1) * 640])
        make_identity(nc, ident)
        nc.vector.memset(ones, 1.0)

        # bf16 cast of x for the G-pass (split ACT/DVE, overlaps the load)
        for k in range(K):
            sl = slice(k * IC, (k + 1) * IC)
            if k % 2 == 0:
                nc.scalar.copy(x2b[:, sl], x2[:, sl])
            else:
                nc.vector.tensor_copy(x2b[:, sl], x2[:, sl])

        # ---- build x1 = per-chunk transpose of x2 (PE transpose) ----
        # evacuation alternates ACT/DVE so neither engine paces the PE
        for t in range(NKT):
            ps = pall[:, t % K, :BL]
            nc.tensor.transpose(ps, x2[:, t * 128:(t + 1) * 128], ident)
            if t % 2 == 0:
                nc.scalar.copy(x1[:, t, :], ps)
            else:
                nc.vector.tensor_copy(x1[:, t, :], ps)

        # iteration 0 has uniform c = 1/IC
        nc.vector.memset(crep[:, :, :], 1.0 / IC)

        bfulls = {}
        for it in range(NUM_ROUTING):
            if it > 0:
                # ---- softmax over i (given b_full from the AllReduce) ----
                b_full = bfulls[it - 1]
                expb = small.tile([128, NT, J], f32, name=f"expb{it}")
                # exp(b/B): fold the batch-mean 1/B into the exp scale
                nc.scalar.activation(
                    expb.rearrange("p t j -> p (t j)"),
                    b_full.rearrange("p t j -> p (t j)"),
                    Act.Exp, scale=1.0 / B)
                # Z[j] = sum_i exp(b[i,j]), broadcast to 128 partitions via
                # an accumulating ones-matmul; bank 7 of PSUM
                zp = pall[:, K - 1, :J]
                for t2 in range(NT):
                    nc.tensor.matmul(zp, ones, expb[:, t2, :],
                                     start=(t2 == 0), stop=(t2 == NT - 1))
                zinv = small.tile([128, J], f32, name=f"zinv{it}")
                nc.vector.reciprocal(zinv, zp)
                # crep[i, (j,u)] = expb[i,j] * zinv[j]  (broadcast over u)
                for t2 in range(NT):
                    nc.vector.tensor_mul(
                        crep[:, t2, :].rearrange("p (j u) -> p j u", j=J),
                        expb[:, t2, :].unsqueeze(-1).broadcast_to([128, J, U]),
                        zinv.unsqueeze(-1).broadcast_to([128, J, U]))

            # ---- wp = crep * W  (72 DVE multiplies) ----
            for t in range(NKT):
                k, t2 = divmod(t, NT)
                nc.vector.tensor_mul(
                    wp[:, t, :].rearrange("p (j u) -> p j u", j=J),
                    w5[:, t2, :, :, k],
                    crep[:, t2, :].rearrange("p (j u) -> p j u", j=J))

            # ---- s = X1^T @ wp : accumulate 72 chunks into PSUM bank 0 ----
            sp = pall[:BL, 0, :JU]
            for t in range(NKT):
                nc.tensor.matmul(sp, x1[:, t, :], wp[:, t, :],
                                 start=(t == 0), stop=(t == NKT - 1))

            # ---- squash (reference quirk: norm over the j axis per (b,u)) ----
            # ACT only does Sqrt here; everything else on DVE to avoid the
            # ~1.3us ACT LUT-table reload per function switch
            s_sb = small.tile([BL, JU], f32, name=f"s_sb{it}")
            nc.vector.tensor_copy(s_sb, sp)
            ssq = small.tile([BL, JU], f32, name=f"ssq{it}")
            nc.vector.tensor_mul(ssq, s_sb, s_sb)
            msq = small.tile([BL, U], f32, name=f"msq{it}")
            nc.vector.tensor_reduce(
                msq, ssq.rearrange("b (j u) -> b u j", j=J),
                axis=mybir.AxisListType.X, op=Alu.add)
            mag = small.tile([BL, U], f32, name=f"mag{it}")
            tpb = small.tile([BL, U], f32, name=f"tpb{it}")
            rin = small.tile([BL, U], f32, name=f"rin{it}")
            fv = small.tile([BL, U], f32, name=f"fv{it}")
            nc.scalar.activation(mag, msq, Act.Sqrt)
            nc.vector.tensor_scalar_add(tpb, msq, BETA)
            nc.vector.reciprocal(rin, tpb)
            nc.vector.tensor_mul(fv, mag, rin)
            v = small.tile([BL, JU], f32, name=f"v{it}")
            nc.vector.tensor_mul(
                v.rearrange("b (j u) -> b j u", j=J),
                s_sb.rearrange("b (j u) -> b j u", j=J),
                fv.unsqueeze(1).broadcast_to([BL, J, U]))

            if it == NUM_ROUTING - 1:
                nc.sync.dma_start(out=out_flat, in_=v)
                continue
            vb = small.tile([BL, JU], bf16, name=f"vb{it}")
            nc.vector.tensor_copy(vb, v)

            # ---- G = X2^T-chunks @ v, per (t2): 8 banks; ACT evacuates the
            # whole PSUM round to SBUF so the next round's matmuls don't
            # serialize behind the DVE b-update reads (PSUM bank hazard) ----
            b_part = small.tile([128, NT, J], f32, name=f"bpart{it}")
            for t2 in range(NT):
                g_sb = scratch.tile([128, K, JU], f32, name="g_sb", bufs=3)
                for h in range(2):
                    for k in range(h * 4, h * 4 + 4):
                        nc.tensor.matmul(
                            pall[:, k, :JU],
                            x2b[:, (k * NT + t2) * 128:
                                (k * NT + t2) * 128 + 128],
                            vb, start=True, stop=True)
                    nc.scalar.copy(g_sb[:, h * 4:h * 4 + 4, :],
                                   pall[:, h * 4:h * 4 + 4, :JU])
                g_fk = g_sb.rearrange("p k f -> p f k")
                for j in range(J):
                    so = scratch.tile([128, U, K], f32, name="stt_scratch")
                    nc.vector.scalar_tensor_tensor(
                        out=so,
                        in0=w5[:, t2, j, :, :],
                        scalar=1.0,
                        in1=g_fk[:, j * U:(j + 1) * U, :],
                        op0=Alu.mult, op1=Alu.mult,
                        accum_out=b_part[:, t2, j:j + 1])

            # ---- AllReduce b over the 8 cores ----
            cc_in = dram.tile([IC, J], f32, name=f"ccin{it}")
            cc_out = dram.tile([IC, J], f32, name=f"ccout{it}",
                               addr_space="Shared")
            cc_in_r = cc_in.rearrange("(t p) j -> p t j", p=128)
            cc_out_r = cc_out.rearrange("(t p) j -> p t j", p=128)
            nc.sync.dma_start(out=cc_in_r, in_=b_part)
            nc.gpsimd.collective_compute(
                "AllReduce", Alu.add,
                replica_groups=[list(range(NCORES))],
                ins=[cc_in[:, :]], outs=[cc_out[:, :]])
            b_full = small.tile([128, NT, J], f32, name=f"bfull{it}")
            nc.sync.dma_start(out=b_full, in_=cc_out_r)
            bfulls[it] = b_full

    nc.compile()
    return nc


def _get_nc():
    if "nc" not in _CACHE:
        _CACHE["nc"] = _build_nc()
    return _CACHE["nc"]


def _run(x, W, trace=False, **kw):
    from concourse import bass_utils

    nc = _get_nc()
    x = np.ascontiguousarray(np.asarray(x, dtype=np.float32))
    W = np.ascontiguousarray(np.asarray(W, dtype=np.float32))
    in_maps = [
        {"xs": x[c * BL:(c + 1) * BL], "w": W}
        for c in range(NCORES)
    ]
    res = bass_utils.run_bass_kernel_spmd(
        nc, in_maps, core_ids=list(range(NCORES)), trace=trace, **kw)
    outs = [res.results[c]["out"] for c in range(NCORES)]
    full = np.concatenate(outs, axis=0).reshape(B, J, 4, U // 4)
    return full, res


def kernel(x, W):
    full, _ = _run(x, W, trace=False)
    return full
